# revision 19
# baseline (speedup 1.0000x reference)
"""Memory-bank attention read on 8 NeuronCores (Trainium2, Bass/Tile).

out[b] = softmax(q_b @ K^T, axis=m) @ K  per batch b, sharded batch->core.

Layout trick: query/output are NCHW, so query[b].reshape(256, 4096) is already
q^T in [d, n] form -- exactly the moving-operand layout the TensorEngine wants.
The whole kernel runs in "transposed" space (scoreT [m, n], outT [d, n]):
  mm1:  scoreT[mi] = kT_chunk.T @ qT_chunk   fp16 operands (1 cyc/row, halves
        the input DMA + H2D bytes vs fp32; score accumulates exactly in PSUM)
  exp:  expT = exp(scoreT - 40) on ScalarE -> bf16 (global shift; bf16 keeps
        fp32's exponent range so e^{score-40} up to ~e^50 cannot overflow)
  mm2:  outT += keys_chunk.T @ expT           bf16 operands
  rs:   Pool+DVE accumulate the 16 exp tiles of a chunk into acc[128, n]
        in fp32 (Pool owns cols 0:PCOL, DVE the rest -- DVE also carries
        recip/muls so Pool gets the bigger share); ONE ones^T @ acc matmul
        per chunk then yields the softmax denominator on every partition.
        expp is 12 deep so exp never waits on the accumulate chain even
        when the chunk-boundary recip/mul burst delays it.
  out:  outT * (1/rowsum) on VectorE -> fp16, one batched DMA per chunk.
"""

import numpy as np
import ml_dtypes

import concourse.bass as bass
import concourse.bacc as bacc
import concourse.mybir as mybir
import concourse.tile as tile
from concourse import bass_utils

B, D, HH, WW = 8, 256, 64, 64
N = HH * WW            # 4096 queries per core
M = 2048               # memory slots
NCH = 512              # n-chunk (1 PSUM bank at fp32)
NCHUNKS = N // NCH     # 8
MT = M // 128          # 16 m-tiles
SHIFT = -40.0          # global softmax shift
PCOL = 224             # Pool's share of the exp-accumulate columns

F32 = mybir.dt.float32
F32R = mybir.dt.float32r
F16 = mybir.dt.float16
BF16 = mybir.dt.bfloat16
NWARM = 6              # PE p-state pre-warm matmuls during the input DMA

_cached_nc = {}


def _build(repeat=1):
    key = repeat
    if key in _cached_nc:
        return _cached_nc[key]

    nc = bacc.Bacc("TRN2", target_bir_lowering=False, debug=False, num_devices=B)
    # all inputs are pre-rearranged on the host into partition-major layout
    # so every DMA moves large contiguous per-partition segments (8KB vs the
    # 512B strides a [m, d] layout would produce)
    qT_d = nc.dram_tensor("qT", [NCHUNKS, 128, 2, NCH], F16, kind="ExternalInput").ap()
    kT_d = nc.dram_tensor("keysT", [128, 2, M], F16, kind="ExternalInput").ap()
    k_d = nc.dram_tensor("keys", [128, MT, D], BF16, kind="ExternalInput").ap()
    o_d = nc.dram_tensor("outT", [NCHUNKS, 128, 2, NCH], F16, kind="ExternalOutput").ap()

    with tile.TileContext(nc) as tc:
        with (
            tc.tile_pool(name="big", bufs=1) as big,
            tc.tile_pool(name="qp", bufs=2) as qp,
            tc.tile_pool(name="expp", bufs=12) as expp,
            tc.tile_pool(name="accp", bufs=2) as accp,
            tc.tile_pool(name="evp", bufs=3) as evp,
            tc.tile_pool(name="ps_s", bufs=3, space=bass.MemorySpace.PSUM) as ps_s,
            tc.tile_pool(name="ps_o", bufs=2, space=bass.MemorySpace.PSUM) as ps_o,
            tc.tile_pool(name="ps_r", bufs=1, space=bass.MemorySpace.PSUM) as ps_r,
        ):
            kT = big.tile([128, 2, M], F16)    # [:, h, :] = keysT rows h*128..
            ks = big.tile([128, MT, D], BF16)  # [:, t, :] = keys rows t*128..
            ones_bf = big.tile([128, 128], BF16)
            ones_fr = big.tile([128, 128], F32R)
            bias = big.tile([128, 1], F32)

            kT_r = kT_d
            ks_r = k_d
            # warm-path constants on Pool (free first), the rest on DVE
            nc.gpsimd.memset(ones_bf[:], 1.0)
            nc.vector.memset(bias[:], SHIFT)
            nc.vector.tensor_copy(ones_fr[:], ones_bf[:])

            # PE p-state pre-warm: the Tensor engine ramps 0.65->1.2->2.4 GHz
            # with continuous work and drops back when idle.  Burn the input
            # DMA wait on dummy matmuls sized to end right as mm1(0)'s
            # operands land, so the real stream starts at full clock.
            warm = big.tile([128, NCH], BF16)
            nc.gpsimd.memset(warm[:], 1.0)
            scr = ps_s.tile([128, NCH], F32, tag="score")
            for _ in range(NWARM):
                nc.tensor.matmul(scr[:], ones_bf[:], warm[:],
                                 start=True, stop=True)

            for rep in range(repeat):
                pending_rs = [None]  # deferred rowsum matmul of previous chunk
                pending_mm2 = []     # previous chunk's trailing mm2 closures

                for nch in range(NCHUNKS):
                    nsl = slice(nch * NCH, (nch + 1) * NCH)
                    qTc = qp.tile([128, 2, NCH], F16, tag="qTc")
                    q_r = qT_d[nch]
                    if rep == 0 and nch == 0:
                        # critical path: q h=0 + kT first tiles feed mm1(0);
                        # kT h0/h1 interleaved in consumption order, ks and
                        # the kT bulk stream in behind
                        # the DMA rings round-robin among active transfers,
                        # so keep the critical qTc alone on Sync's ring and
                        # put kT/ks on Scalar's (also a HWDGE engine, idle
                        # until exp(0)).  Each dma_start costs ~800ns of
                        # serial issue time on its engine; the far tail
                        # pieces are deferred to chunk 1's emission.
                        nc.scalar.dma_start(qTc[:], q_r)
                        nc.scalar.dma_start(kT[:, :, 0:512], kT_r[:, :, 0:512])
                        nc.scalar.dma_start(ks[:, 0:4, :], ks_r[:, 0:4, :])
                        nc.scalar.dma_start(kT[:, :, 512:1280],
                                            kT_r[:, :, 512:1280])
                        nc.scalar.dma_start(ks[:, 4:10, :], ks_r[:, 4:10, :])
                        nc.scalar.dma_start(kT[:, :, 1280:M],
                                            kT_r[:, :, 1280:M])
                        nc.scalar.dma_start(ks[:, 10:MT, :], ks_r[:, 10:MT, :])
                    else:
                        nc.sync.dma_start(qTc[:], q_r)
                    out0 = ps_o.tile([128, NCH], F32, tag="out0")
                    out1 = ps_o.tile([128, NCH], F32, tag="out1")
                    # exp-tile accumulator: Pool owns cols 0:PCOL, DVE owns
                    # PCOL:512 -- two otherwise-idle engines, no merge step
                    acc = accp.tile([128, NCH], F32R, tag="acc")
                    expts = [None] * MT
                    scores = [None] * MT

                    def mm1(i):
                        sc = ps_s.tile([128, NCH], F32, tag="score")
                        for h in range(2):
                            nc.tensor.matmul(
                                sc[:],
                                kT[:, h, i * 128:(i + 1) * 128],
                                qTc[:, h, :],
                                start=(h == 0),
                                stop=(h == 1),
                            )
                        scores[i] = sc

                    last = nch == NCHUNKS - 1

                    def do_exp(i):
                        e = expp.tile([128, NCH], BF16, tag="expt")
                        nc.scalar.activation(
                            e[:], scores[i][:], mybir.ActivationFunctionType.Exp,
                            bias=bias[:], scale=1.0,
                        )
                        expts[i] = e
                        if last and i >= MT - 2:
                            # final chunk: last two tiles join the rowsum via
                            # direct PE matmuls, off the accumulator chain, so
                            # the drain never waits on the SIMD engines
                            return
                        for eng, cs in ((nc.gpsimd, slice(0, PCOL)),
                                        (nc.vector, slice(PCOL, NCH))):
                            if i == 0:
                                eng.tensor_copy(acc[:, cs], e[:, cs])
                            else:
                                eng.tensor_add(acc[:, cs], acc[:, cs], e[:, cs])

                    def mm2(i, out0=out0, out1=out1, expts=expts):
                        e = expts[i][:]
                        st, sp = (i == 0), (i == MT - 1)
                        nc.tensor.matmul(out0[:], ks[:, i, 0:128], e,
                                         start=st, stop=sp)
                        nc.tensor.matmul(out1[:], ks[:, i, 128:256], e,
                                         start=st, stop=sp)

                    for i in range(MT):
                        # previous chunk's trailing mm2 first: its exp tile is
                        # long ready, so it absorbs any boundary hiccup
                        if i < 2 and pending_mm2:
                            pending_mm2.pop(0)()
                        mm1(i)
                        # rowsum matmul of the PREVIOUS chunk: emitted early in
                        # this chunk's PE stream so the engine never stalls on
                        # the Pool accumulator finishing at a chunk boundary.
                        if i == 6 and pending_rs[0] is not None:
                            pending_rs[0]()
                            pending_rs[0] = None
                        do_exp(i)
                        # mm2 runs two tiles behind mm1, rolling ACROSS chunk
                        # boundaries so the pipeline never refills from empty
                        if i >= 2:
                            mm2(i - 2)

                    def finish(acc=acc, out0=out0, out1=out1, nch_=nch):
                        # fp32r moving runs 1 cyc/row at this width, so the
                        # rowsum matmul reads the fp32 accumulator directly --
                        # no bf16 rounding pass on the SIMD engines at all
                        rs = ps_r.tile([128, NCH], F32, tag="rs")
                        nc.tensor.matmul(
                            rs[:], ones_fr[:], acc[:],
                            start=True, stop=True,
                        )
                        recip = evp.tile([128, NCH], F32, tag="recip")
                        o01 = evp.tile([128, 2, NCH], F16, tag="o01")
                        nc.vector.reciprocal_approx_fast(recip[:], rs[:])
                        nc.vector.tensor_mul(o01[:, 0, :], out0[:], recip[:])
                        nc.vector.tensor_mul(o01[:, 1, :], out1[:], recip[:])
                        nc.sync.dma_start(o_d[nch_], o01[:])

                    if not last:
                        pending_mm2[:] = [
                            lambda f=mm2: f(MT - 2),
                            lambda f=mm2: f(MT - 1),
                        ]
                        pending_rs[0] = finish
                        continue

                    # final chunk drain: rowsum = ones @ (acc + e14 + e15);
                    # the partials land on PE between the trailing mm2s so
                    # nothing waits on the SIMD engines.
                    rs = ps_r.tile([128, NCH], F32, tag="rs")
                    mm2(MT - 2)
                    nc.tensor.matmul(rs[:], ones_bf[:], expts[MT - 2][:],
                                     start=True, stop=False)
                    nc.tensor.matmul(rs[:], ones_fr[:], acc[:],
                                     start=False, stop=False)
                    nc.tensor.matmul(rs[:], ones_bf[:], expts[MT - 1][:],
                                     start=False, stop=True)
                    mm2(MT - 1)
                    recip = evp.tile([128, NCH], F32, tag="recip")
                    o01 = evp.tile([128, 2, NCH], F16, tag="o01")
                    nc.vector.reciprocal_approx_fast(recip[:], rs[:])
                    o_r = o_d[nch]
                    # split DMA so out0 ships while out1 is still scaling
                    nc.vector.tensor_mul(o01[:, 0, :], out0[:], recip[:])
                    nc.sync.dma_start(o_r[:, 0, :], o01[:, 0, :])
                    nc.vector.tensor_mul(o01[:, 1, :], out1[:], recip[:])
                    nc.sync.dma_start(o_r[:, 1, :], o01[:, 1, :])

    nc.compile()
    _cached_nc[key] = nc
    return nc


def _in_maps(keys, query):
    keys = np.asarray(keys, dtype=np.float32)
    q = np.asarray(query, dtype=np.float32)
    # partition-major relayouts so DMA descriptors are large and contiguous:
    # kT16 [p, h, m]; kb16 [p, t, d]; q16 [b, chunk, p, h, n-in-chunk]
    kT16 = np.ascontiguousarray(
        keys.T.astype(np.float16).reshape(2, 128, M).transpose(1, 0, 2))
    kb16 = np.ascontiguousarray(
        keys.astype(ml_dtypes.bfloat16).reshape(MT, 128, D).transpose(1, 0, 2))
    q16 = np.ascontiguousarray(
        q.reshape(B, 2, 128, NCHUNKS, NCH).astype(np.float16)
        .transpose(0, 3, 2, 1, 4))
    return [
        {"qT": q16[b], "keysT": kT16, "keys": kb16}
        for b in range(B)
    ]


def _run(keys, query, trace=False, repeat=1, **trace_kwargs):
    nc = _build(repeat)
    return bass_utils.run_bass_kernel_spmd(
        nc, _in_maps(keys, query), core_ids=list(range(B)), trace=trace,
        **trace_kwargs
    )


def kernel(keys, query, value):
    res = _run(keys, query)
    # outT per core is [chunk, p, h, n-in-chunk]; undo the relayout
    out = np.stack([res.results[b]["outT"] for b in range(B)])
    out = out.astype(np.float32).transpose(0, 3, 2, 1, 4).reshape(B, D, N)
    return np.ascontiguousarray(out.reshape(B, D, HH, WW))
